# revision 34
# baseline (speedup 1.0000x reference)
"""Trainium2 Bass kernel for the GraphicalBranch GNN message-passing problem.

Math (equivalent to the reference):
  - Per-sample graphs are fully connected WITH self-loops over the nc2=28
    pair-nodes, so segment_sum(x[src], dst) == broadcast of the per-sample
    row-sum S[b] = sum_r x[b, r, :].
  - The final key-matching gather h[rows] commutes with the row-wise linear
    layer, so only the 10 gathered rows per sample are pushed through W_self:
        out[b*10+k] = relu(xg[b*10+k] @ W_self + (S[b] @ W_nbr + b))
  - rows computed on host from slicing_tensor/object_pairs (index arithmetic).

Sharding: data-parallel over samples; each of 8 cores gets 128 samples
(3584 x-rows, 1280 output rows). Weights replicated.

Per-core schedule:
  - x (bf16) streams in 4 chunks of 896 rows (=32 samples); chunks 0/3 are
    split so the PE starts early and the tail starts early.  Chunk c's 7
    G-matmuls accumulate S rows [32c,32c+32) in one PSUM bank.
  - xg @ W_self runs in fp8 e4m3 with perf_mode=DoubleRow (2 k-subtiles per
    matmul).  ws+xgT are packed in ONE dram tensor with 7KB partition lines
    (small-line DMAs measured at ~170 B/ns vs ~420 B/ns for 7KB lines).
    The S/A path stays bf16 (fp8 there fails the 2e-2 gate).
  - A = S @ W_nbr + b lands in three pieces: samples 0-63 after chunk 1,
    64-95 after chunk 2, 96-127 after chunk 3 (psS slice -> DVE copy ->
    PE transposes -> k-matmuls into psA, partition-aligned with abf).
    Output tile t references samples [12.8t, 12.8t+12.7]: tiles 0-4 close on
    A(0-63), 5-6 on A(64-95), 7-9 on A(96-127) -- only 3 closes depend on
    the last x chunk.  abf[1] rows 32-63 are memset 0 so tiles 5-6 can
    contract K=65 before A-q3 lands (their eT rows there are zero).
  - Each W_self group closes with ONE K=65 expansion matmul: lhsT = per-half
    one-hot eTh (64 sample rows + ones row), rhs = abf (A rows + bias row),
    adding E@A AND the bias in one pass.
  - ReLUs alternate ScalarE activation / DVE tensor_scalar_max to halve the
    serial epilogue; paired bf16 stores on the sync ring after all load
    issues.  Host upcasts bf16 -> f32.

PSUM: psS + psT + psA + 5 live out-groups = 8 banks.  PE transposes are
interleaved with group opens so the psT round-trip latency is hidden.
"""

import numpy as np
import ml_dtypes

# ---- problem constants (hardcoded; kernel.py must be self-contained) ----
B = 1024          # samples
NOBJ = 8          # objects per sample
NC2 = 28          # pair-nodes per sample
MAXR = 10         # relations per sample
D = 512           # feature dim
NCORES = 8
BL = B // NCORES          # 128 samples per core
RL = BL * NC2             # 3584 x-rows per core
ML = BL * MAXR            # 1280 output rows per core
KT = D // 128             # 4 contraction tiles
MT = ML // 128            # 10 output row tiles per core
XCH = 4                   # x chunks (896 rows = 32 samples each)
RJ = 7                    # row-tiles per chunk
SW = BL // XCH            # 32 samples per chunk
HM = ML // 2              # 640 output rows per half
GW = D + MT * 128         # 1792: packed ws||xgT row elems per k-tile

BF16 = ml_dtypes.bfloat16
FP8 = ml_dtypes.float8_e4m3

_compiled = None


def _build_bass():
    import concourse.bacc as bacc
    import concourse.bass as bass
    import concourse.mybir as mybir
    from concourse import tile

    f32 = mybir.dt.float32
    bf16 = mybir.dt.bfloat16
    fp8 = mybir.dt.float8e4
    DR = mybir.MatmulPerfMode.DoubleRow

    nc = bacc.Bacc("TRN2", target_bir_lowering=False, debug=False,
                   num_devices=NCORES)

    # host-prelaid, partition-major contiguous inputs
    x_d = nc.dram_tensor("x", [XCH, 128, RJ * D], bf16, kind="ExternalInput")
    g_d = nc.dram_tensor("g", [128, RJ * SW], bf16, kind="ExternalInput")
    # big8[p, kt, 0:512] = W_self[kt*128+p, :]; [p, kt, 512+t*128+m] =
    #   xg[t*128+m, kt*128+p]  (fp8, 7168B lines)
    big8_d = nc.dram_tensor("big8", [128, KT * GW], fp8, kind="ExternalInput")
    id_d = nc.dram_tensor("ident", [128, 128], bf16, kind="ExternalInput")
    wn_d = nc.dram_tensor("wn", [128, KT * D], bf16, kind="ExternalInput")
    # per-half expansion blocks: 64 one-hot sample rows + ones row (bias)
    eT_d = nc.dram_tensor("eTh", [65, 2 * HM], bf16, kind="ExternalInput")
    b_d = nc.dram_tensor("bias", [1, D], bf16, kind="ExternalInput")
    # out[p, t*D:(t+1)*D] = out-row (t*128+p): 5KB partition lines per
    # half-store (the row-pair layout had 1KB lines -> ~170 B/ns stores)
    out_d = nc.dram_tensor("out", [128, MT * D], bf16, kind="ExternalOutput")

    with tile.TileContext(nc) as tc:
        with (
            tc.tile_pool(name="const", bufs=1) as cpool,
            tc.tile_pool(name="psum", bufs=4, space=bass.MemorySpace.PSUM) as ppool,
            tc.tile_pool(name="psumS", bufs=1, space=bass.MemorySpace.PSUM) as pspool,
            tc.tile_pool(name="psumT", bufs=1, space=bass.MemorySpace.PSUM) as ptpool,
            tc.tile_pool(name="psumA", bufs=1, space=bass.MemorySpace.PSUM) as papool,
        ):
            # abf_h: rows 0-63 = A for samples [64h, 64h+64), row 64 = bias
            abf = [cpool.tile([65, D], bf16, name=f"abf{h}", tag=f"abf{h}")
                   for h in range(2)]
            # tiles 5-6 contract abf[1] rows 32-63 (zero eT cols) before A-q3
            nc.gpsimd.memset(abf[1][32:64, :], 0.0)

            # ---- scalar-ring loads: small only (the scalar ring is starved
            # ---- to ~50 B/ns while the sync ring streams -- id/wn go on
            # ---- the sync ring instead, sequenced where they are needed)
            g_sb = cpool.tile([128, RJ, SW], bf16)
            nc.scalar.dma_start(g_sb[:], g_d.rearrange("p (j s) -> p j s", s=SW))
            eT_sb = cpool.tile([65, 2 * HM], bf16)
            nc.scalar.dma_start(eT_sb[:], eT_d[:, :])
            nc.scalar.dma_start(abf[0][64:65, :], b_d[:, :])
            nc.scalar.dma_start(abf[1][64:65, :], b_d[:, :])
            id_sb = cpool.tile([128, 128], bf16)
            wn_sb = cpool.tile([128, KT * D], bf16)

            def id_s(lo, hi):            # identity block rows/cols [lo:hi)
                return id_sb[lo:hi, lo:hi]

            def wn_s(kt):                # W_nbr k-tile [128, 512]
                return wn_sb[:, kt * D:(kt + 1) * D]

            def eT_s(h, tl):             # expansion block [65, 128]
                return eT_sb[:, h * HM + tl * 128:h * HM + (tl + 1) * 128]

            # ---- sync-ring loads: x0a x0b big8 x1 x2 x3a x3b ----
            xch = [cpool.tile([128, RJ, D], bf16, name=f"x{c}", tag=f"x{c}")
                   for c in range(XCH)]
            big8_sb = cpool.tile([128, KT, GW], fp8)
            x_r = [x_d[c].rearrange("p (j d) -> p j d", d=D) for c in range(XCH)]

            nc.sync.dma_start(xch[0][:, 0:3], x_r[0][:, 0:3])
            nc.sync.dma_start(xch[0][:, 3:RJ], x_r[0][:, 3:RJ])
            nc.sync.dma_start(id_sb[:], id_d[:, :])
            nc.sync.dma_start(xch[1][:], x_r[1])
            nc.sync.dma_start(big8_sb[:],
                              big8_d.rearrange("p (k w) -> p k w", w=GW))
            nc.sync.dma_start(wn_sb[:], wn_d[:, :])
            nc.sync.dma_start(xch[2][:], x_r[2])
            nc.sync.dma_start(xch[3][:, 0:4], x_r[3][:, 0:4])
            nc.sync.dma_start(xch[3][:, 4:RJ], x_r[3][:, 4:RJ])

            # ---- compute ----
            psS = pspool.tile([128, D], f32)
            s_bf = cpool.tile([128, KT, BL], bf16)     # S^T (lhsT for A)
            snat = cpool.tile([128, D], bf16)          # psS copy staging
            psA = papool.tile([128, D], f32)
            out_r = out_d.rearrange("p (t n) -> p t n", n=D)
            oth = [cpool.tile([128, 5, D], bf16, name=f"oth{h}", tag=f"oth{h}")
                   for h in range(2)]

            def s_js(c, js):
                for j in js:
                    nc.tensor.matmul(psS[c * SW:(c + 1) * SW, :],
                                     g_sb[:, j, :], xch[c][:, j, :],
                                     start=(j == 0), stop=(j == RJ - 1),
                                     tile_position=(0, c * SW))

            def s_chunk(c):
                s_js(c, range(RJ))

            main_ps = {}

            def open_group(t):
                ps = ppool.tile([128, D], f32, tag="ps")
                for kp in range(KT // 2):
                    nc.tensor.matmul(ps[:],
                                     big8_sb[:, 2 * kp:2 * kp + 2,
                                             D + t * 128:D + (t + 1) * 128],
                                     big8_sb[:, 2 * kp:2 * kp + 2, 0:D],
                                     start=(kp == 0), stop=False,
                                     perf_mode=DR)
                main_ps[t] = ps

            def close_group(t):
                h, tl = t // 5, t % 5
                ps = main_ps.pop(t)
                nc.tensor.matmul(ps[:], eT_s(h, tl), abf[h][:, :],
                                 start=False, stop=True)
                if t % 2 == 0:
                    nc.scalar.activation(oth[h][:, tl, :], ps[:],
                                         mybir.ActivationFunctionType.Relu)
                else:
                    nc.vector.tensor_scalar_max(oth[h][:, tl, :], ps[:], 0.0)
                # h0 stores once (overlapped); h1 in pieces so the epilogue
                # only waits for the last 128-row store
                if t == 4:
                    nc.sync.dma_start(out_r[:, 0:5], oth[0][:])
                elif t == 6:
                    nc.sync.dma_start(out_r[:, 5:7], oth[1][:, 0:2])
                elif t == 8:
                    nc.sync.dma_start(out_r[:, 7:9], oth[1][:, 2:4])
                elif t == 9:
                    nc.sync.dma_start(out_r[:, 9:10], oth[1][:, 4:5])

            def a_mat(lo, hi, kt):
                alo, n = lo % 64, hi - lo
                nc.tensor.matmul(psA[alo:alo + n, :], s_bf[:, kt, lo:hi],
                                 wn_s(kt),
                                 start=(kt == 0), stop=(kt == KT - 1),
                                 tile_position=(0, alo))

            def a_piece(lo, hi):
                """A rows for samples [lo,hi) -> psA -> abf.  Transposes and
                A k-matmuls in tight same-type runs (type-interleaving was
                measured to hold the PE at low clock); double-buffered psT."""
                n, h = hi - lo, lo // 64
                nc.vector.tensor_copy(snat[lo:hi, :], psS[lo:hi, :])
                for dt in range(KT):
                    psT = ptpool.tile([128, 64], bf16, tag=f"psT{dt % 2}")
                    nc.tensor.transpose(psT[:, 0:n],
                                        snat[lo:hi, dt * 128:(dt + 1) * 128],
                                        id_s(lo, hi),
                                        tile_position=(lo, 0))
                    nc.vector.tensor_copy(s_bf[:, dt, lo:hi], psT[:, 0:n])
                for kt in range(KT):
                    a_mat(lo, hi, kt)
                alo = lo % 64
                nc.vector.tensor_copy(abf[h][alo:alo + n, :],
                                      psA[alo:alo + n, :])

            s_chunk(0)
            s_chunk(1)
            for t in range(4):      # group 4 opens after close 0 frees a bank
                open_group(t)
            a_piece(0, 64)          # A samples 0-63 (only needs chunks 0-1)
            # tiles 0-4 touch only samples 0-63; each close frees the PSUM
            # bank the following open reuses
            close_group(0)
            open_group(4)
            close_group(1)
            open_group(5)
            close_group(2)
            open_group(6)
            close_group(3)
            open_group(7)
            close_group(4)
            open_group(8)
            s_chunk(2)
            # A samples 64-95; tiles 5-6 only touch samples 64-89
            a_piece(64, 96)
            close_group(5)
            open_group(9)
            close_group(6)
            s_chunk(3)
            a_piece(96, 128)        # A samples 96-127
            for t in range(7, 10):
                close_group(t)

    nc.compile()
    return nc


def _get_compiled():
    global _compiled
    if _compiled is None:
        _compiled = _build_bass()
    return _compiled


def _host_prep(inputs):
    """Shard + preprocess on host. Returns per-core input maps."""
    x = np.asarray(inputs["spatial_branch_feature_map"], dtype=np.float32)
    W_self = np.asarray(inputs["W_self"], dtype=np.float32)
    W_nbr = np.asarray(inputs["W_nbr"], dtype=np.float32)
    b = np.asarray(inputs["b"], dtype=np.float32)
    st = np.asarray(inputs["slicing_tensor"])
    op = np.asarray(inputs["object_pairs"])

    N = x.shape[0]
    n = NOBJ
    # exact replication of the reference's LUT-based row computation
    keys = st[:, 0].astype(np.int64) * (n * n) + st[:, 1].astype(np.int64) * n \
        + st[:, 2].astype(np.int64)
    lut = np.zeros(B * n * n, dtype=np.int64)
    lut[keys] = np.arange(N, dtype=np.int64)
    pmin = np.minimum(op[..., 0], op[..., 1]).astype(np.int64)
    pmax = np.maximum(op[..., 0], op[..., 1]).astype(np.int64)
    rel_keys = (np.arange(B, dtype=np.int64)[:, None] * (n * n)
                + pmin * n + pmax).reshape(-1)
    rows = lut[rel_keys]                      # [B*MAXR] global row index

    xg = x[rows]                              # [B*MAXR, D]
    # x: [NCORES, XCH, 128, RJ*D]; sbuf[p, j, :] = x_core[ch*896 + j*128 + p]
    x_bf = np.ascontiguousarray(
        x.astype(BF16).reshape(NCORES, XCH, RJ, 128, D)
        .transpose(0, 1, 3, 2, 4).reshape(NCORES, XCH, 128, RJ * D))
    # packed fp8 ws||xgT: big8[c][p, kt, 0:512] = W_self[kt*128+p, :],
    #   big8[c][p, kt, 512+t*128+m] = xg_c[t*128+m, kt*128+p]
    ws8 = np.ascontiguousarray(
        W_self.astype(FP8).reshape(KT, 128, D).transpose(1, 0, 2))  # [128,KT,D]
    xgT8 = (xg.astype(FP8).reshape(NCORES, MT, 128, KT, 128)
            .transpose(0, 4, 3, 1, 2))       # [NCORES, 128, KT, MT, 128]
    big8 = np.empty((NCORES, 128, KT, GW), dtype=FP8)
    big8[:, :, :, 0:D] = ws8[None]
    big8[:, :, :, D:] = xgT8.reshape(NCORES, 128, KT, MT * 128)
    big8 = big8.reshape(NCORES, 128, KT * GW)

    wn = np.ascontiguousarray(
        W_nbr.astype(BF16).reshape(KT, 128, D).transpose(1, 0, 2)
        .reshape(128, KT * D))
    ident = np.eye(128, dtype=BF16)

    # eTh[i<64, h*HM + m] = ((640h + m)//10 == 64h + i); row 64 = 1 (bias)
    eTh = np.zeros((65, 2 * HM), dtype=BF16)
    for h in range(2):
        m = np.arange(HM) + h * HM
        eTh[:64, h * HM:(h + 1) * HM] = (
            (m[None, :] // MAXR) == (np.arange(64)[:, None] + 64 * h)
        ).astype(BF16)
    eTh[64, :] = BF16(1.0)
    # shared one-hot block: g[p, j*SW + s] = ((j*128 + p)//NC2 == s)
    jj = np.arange(RJ * 128)
    g = (jj[:, None] // NC2 == np.arange(SW)[None, :]).astype(BF16)
    g = np.ascontiguousarray(
        g.reshape(RJ, 128, SW).transpose(1, 0, 2).reshape(128, RJ * SW))
    bias = b.astype(BF16).reshape(1, D)

    in_maps = []
    for c in range(NCORES):
        in_maps.append({
            "x": x_bf[c], "big8": big8[c], "g": g,
            "ident": ident, "wn": wn, "eTh": eTh, "bias": bias,
        })
    return in_maps


def run(inputs, trace=False):
    """Returns (full_output, BassKernelResults)."""
    from concourse.bass_utils import run_bass_kernel_spmd

    nc = _get_compiled()
    in_maps = _host_prep(inputs)
    res = run_bass_kernel_spmd(nc, in_maps, core_ids=list(range(NCORES)),
                               trace=trace)
    outs = []
    for r in res.results:
        o = np.asarray(r["out"]).reshape(128, MT, D)
        outs.append(o.transpose(1, 0, 2).reshape(ML, D))
    return np.concatenate(outs, axis=0).astype(np.float32), res


def kernel(**inputs) -> np.ndarray:
    out, _ = run(inputs, trace=False)
    return out


# revision 35
# speedup vs baseline: 1.0164x; 1.0164x over previous
"""Trainium2 Bass kernel for the GraphicalBranch GNN message-passing problem.

Math (equivalent to the reference):
  - Per-sample graphs are fully connected WITH self-loops over the nc2=28
    pair-nodes, so segment_sum(x[src], dst) == broadcast of the per-sample
    row-sum S[b] = sum_r x[b, r, :].
  - The final key-matching gather h[rows] commutes with the row-wise linear
    layer, so only the 10 gathered rows per sample are pushed through W_self:
        out[b*10+k] = relu(xg[b*10+k] @ W_self + (S[b] @ W_nbr + b))
  - rows computed on host from slicing_tensor/object_pairs (index arithmetic).

Sharding: data-parallel over samples; each of 8 cores gets 128 samples
(3584 x-rows, 1280 output rows). Weights replicated.

Per-core schedule:
  - x (bf16) streams in 4 chunks of 896 rows (=32 samples); chunks 0/3 are
    split so the PE starts early and the tail starts early.  Chunk c's 7
    G-matmuls accumulate S rows [32c,32c+32) in one PSUM bank.
  - xg @ W_self runs in fp8 e4m3 with perf_mode=DoubleRow (2 k-subtiles per
    matmul).  ws+xgT are packed in ONE dram tensor with 7KB partition lines
    (small-line DMAs measured at ~170 B/ns vs ~420 B/ns for 7KB lines).
    The S/A path stays bf16 (fp8 there fails the 2e-2 gate).
  - A = S @ W_nbr + b lands in three pieces: samples 0-63 after chunk 1,
    64-95 after chunk 2, 96-127 after chunk 3 (psS slice -> DVE copy ->
    PE transposes -> k-matmuls into psA, partition-aligned with abf).
    Output tile t references samples [12.8t, 12.8t+12.7]: tiles 0-4 close on
    A(0-63), 5-6 on A(64-95), 7-9 on A(96-127) -- only 3 closes depend on
    the last x chunk.  abf[1] rows 32-63 are memset 0 so tiles 5-6 can
    contract K=65 before A-q3 lands (their eT rows there are zero).
  - Each W_self group closes with ONE K=65 expansion matmul: lhsT = per-half
    one-hot eTh (64 sample rows + ones row), rhs = abf (A rows + bias row),
    adding E@A AND the bias in one pass.
  - ReLUs alternate ScalarE activation / DVE tensor_scalar_max to halve the
    serial epilogue; paired bf16 stores on the sync ring after all load
    issues.  Host upcasts bf16 -> f32.

PSUM: psS + psT + psA + 5 live out-groups = 8 banks.  PE transposes are
interleaved with group opens so the psT round-trip latency is hidden.
"""

import numpy as np
import ml_dtypes

# ---- problem constants (hardcoded; kernel.py must be self-contained) ----
B = 1024          # samples
NOBJ = 8          # objects per sample
NC2 = 28          # pair-nodes per sample
MAXR = 10         # relations per sample
D = 512           # feature dim
NCORES = 8
BL = B // NCORES          # 128 samples per core
RL = BL * NC2             # 3584 x-rows per core
ML = BL * MAXR            # 1280 output rows per core
KT = D // 128             # 4 contraction tiles
MT = ML // 128            # 10 output row tiles per core
XCH = 4                   # x chunks (896 rows = 32 samples each)
RJ = 7                    # row-tiles per chunk
SW = BL // XCH            # 32 samples per chunk
HM = ML // 2              # 640 output rows per half
GW = D + MT * 128         # 1792: packed ws||xgT row elems per k-tile

BF16 = ml_dtypes.bfloat16
FP8 = ml_dtypes.float8_e4m3

_compiled = None


def _build_bass():
    import concourse.bacc as bacc
    import concourse.bass as bass
    import concourse.mybir as mybir
    from concourse import tile

    f32 = mybir.dt.float32
    bf16 = mybir.dt.bfloat16
    fp8 = mybir.dt.float8e4
    DR = mybir.MatmulPerfMode.DoubleRow

    nc = bacc.Bacc("TRN2", target_bir_lowering=False, debug=False,
                   num_devices=NCORES)

    # host-prelaid, partition-major contiguous inputs
    x_d = nc.dram_tensor("x", [XCH, 128, RJ * D], bf16, kind="ExternalInput")
    g_d = nc.dram_tensor("g", [128, RJ * SW], bf16, kind="ExternalInput")
    # big8[p, kt, 0:512] = W_self[kt*128+p, :]; [p, kt, 512+t*128+m] =
    #   xg[t*128+m, kt*128+p]  (fp8, 7168B lines)
    big8_d = nc.dram_tensor("big8", [128, KT * GW], fp8, kind="ExternalInput")
    id_d = nc.dram_tensor("ident", [128, 128], bf16, kind="ExternalInput")
    wn_d = nc.dram_tensor("wn", [128, KT * D], bf16, kind="ExternalInput")
    # per-half expansion blocks: 64 one-hot sample rows + ones row (bias)
    eT_d = nc.dram_tensor("eTh", [65, 2 * HM], bf16, kind="ExternalInput")
    b_d = nc.dram_tensor("bias", [1, D], bf16, kind="ExternalInput")
    # out[p, t*D:(t+1)*D] = out-row (t*128+p): 5KB partition lines per
    # half-store (the row-pair layout had 1KB lines -> ~170 B/ns stores)
    out_d = nc.dram_tensor("out", [128, MT * D], bf16, kind="ExternalOutput")

    with tile.TileContext(nc) as tc:
        with (
            tc.tile_pool(name="const", bufs=1) as cpool,
            tc.tile_pool(name="psum", bufs=4, space=bass.MemorySpace.PSUM) as ppool,
            tc.tile_pool(name="psumS", bufs=1, space=bass.MemorySpace.PSUM) as pspool,
            tc.tile_pool(name="psumT", bufs=1, space=bass.MemorySpace.PSUM) as ptpool,
            tc.tile_pool(name="psumA", bufs=1, space=bass.MemorySpace.PSUM) as papool,
        ):
            # abf_h: rows 0-63 = A for samples [64h, 64h+64), row 64 = bias
            abf = [cpool.tile([65, D], bf16, name=f"abf{h}", tag=f"abf{h}")
                   for h in range(2)]
            # tiles 5-6 contract abf[1] rows 32-63 (zero eT cols) before A-q3
            nc.gpsimd.memset(abf[1][32:64, :], 0.0)

            # ---- scalar-ring loads: small only (the scalar ring is starved
            # ---- to ~50 B/ns while the sync ring streams -- id/wn go on
            # ---- the sync ring instead, sequenced where they are needed)
            g_sb = cpool.tile([128, RJ, SW], bf16)
            nc.scalar.dma_start(g_sb[:], g_d.rearrange("p (j s) -> p j s", s=SW))
            eT_sb = cpool.tile([65, 2 * HM], bf16)
            nc.scalar.dma_start(eT_sb[:], eT_d[:, :])
            nc.scalar.dma_start(abf[0][64:65, :], b_d[:, :])
            nc.scalar.dma_start(abf[1][64:65, :], b_d[:, :])
            id_sb = cpool.tile([128, 128], bf16)
            wn_sb = cpool.tile([128, KT * D], bf16)

            def id_s(lo, hi):            # identity block rows/cols [lo:hi)
                return id_sb[lo:hi, lo:hi]

            def wn_s(kt):                # W_nbr k-tile [128, 512]
                return wn_sb[:, kt * D:(kt + 1) * D]

            def eT_s(h, tl):             # expansion block [65, 128]
                return eT_sb[:, h * HM + tl * 128:h * HM + (tl + 1) * 128]

            # ---- sync-ring loads: x0a x0b big8 x1 x2 x3a x3b ----
            xch = [cpool.tile([128, RJ, D], bf16, name=f"x{c}", tag=f"x{c}")
                   for c in range(XCH)]
            big8_sb = cpool.tile([128, KT, GW], fp8)
            x_r = [x_d[c].rearrange("p (j d) -> p j d", d=D) for c in range(XCH)]

            nc.sync.dma_start(xch[0][:, 0:4], x_r[0][:, 0:4])
            nc.sync.dma_start(xch[0][:, 4:RJ], x_r[0][:, 4:RJ])
            nc.sync.dma_start(id_sb[:], id_d[:, :])
            nc.sync.dma_start(big8_sb[:],
                              big8_d.rearrange("p (k w) -> p k w", w=GW))
            nc.sync.dma_start(xch[1][:], x_r[1])
            nc.sync.dma_start(wn_sb[:], wn_d[:, :])
            nc.sync.dma_start(xch[2][:], x_r[2])
            nc.sync.dma_start(xch[3][:, 0:4], x_r[3][:, 0:4])
            nc.sync.dma_start(xch[3][:, 4:RJ], x_r[3][:, 4:RJ])

            # ---- compute ----
            psS = pspool.tile([128, D], f32)
            s_bf = cpool.tile([128, KT, BL], bf16)     # S^T (lhsT for A)
            snat = cpool.tile([128, D], bf16)          # psS copy staging
            psA = papool.tile([128, D], f32)
            out_r = out_d.rearrange("p (t n) -> p t n", n=D)
            oth = [cpool.tile([128, 5, D], bf16, name=f"oth{h}", tag=f"oth{h}")
                   for h in range(2)]

            def s_js(c, js):
                for j in js:
                    nc.tensor.matmul(psS[c * SW:(c + 1) * SW, :],
                                     g_sb[:, j, :], xch[c][:, j, :],
                                     start=(j == 0), stop=(j == RJ - 1),
                                     tile_position=(0, c * SW))

            def s_chunk(c):
                s_js(c, range(RJ))

            main_ps = {}

            def open_group(t):
                ps = ppool.tile([128, D], f32, tag="ps")
                for kp in range(KT // 2):
                    nc.tensor.matmul(ps[:],
                                     big8_sb[:, 2 * kp:2 * kp + 2,
                                             D + t * 128:D + (t + 1) * 128],
                                     big8_sb[:, 2 * kp:2 * kp + 2, 0:D],
                                     start=(kp == 0), stop=False,
                                     perf_mode=DR)
                main_ps[t] = ps

            def close_group(t):
                h, tl = t // 5, t % 5
                ps = main_ps.pop(t)
                nc.tensor.matmul(ps[:], eT_s(h, tl), abf[h][:, :],
                                 start=False, stop=True)
                if t % 2 == 0:
                    nc.scalar.activation(oth[h][:, tl, :], ps[:],
                                         mybir.ActivationFunctionType.Relu)
                else:
                    nc.vector.tensor_scalar_max(oth[h][:, tl, :], ps[:], 0.0)
                # h0 stores once (overlapped); h1 in pieces so the epilogue
                # only waits for the last 128-row store
                if t == 4:
                    nc.sync.dma_start(out_r[:, 0:5], oth[0][:])
                elif t == 6:
                    nc.sync.dma_start(out_r[:, 5:7], oth[1][:, 0:2])
                elif t == 8:
                    nc.sync.dma_start(out_r[:, 7:9], oth[1][:, 2:4])
                elif t == 9:
                    nc.sync.dma_start(out_r[:, 9:10], oth[1][:, 4:5])

            def a_mat(lo, hi, kt):
                alo, n = lo % 64, hi - lo
                nc.tensor.matmul(psA[alo:alo + n, :], s_bf[:, kt, lo:hi],
                                 wn_s(kt),
                                 start=(kt == 0), stop=(kt == KT - 1),
                                 tile_position=(0, alo))

            def a_piece(lo, hi):
                """A rows for samples [lo,hi) -> psA -> abf.  Transposes and
                A k-matmuls in tight same-type runs (type-interleaving was
                measured to hold the PE at low clock); double-buffered psT."""
                n, h = hi - lo, lo // 64
                nc.vector.tensor_copy(snat[lo:hi, :], psS[lo:hi, :])
                for dt in range(KT):
                    psT = ptpool.tile([128, 64], bf16, tag=f"psT{dt % 2}")
                    nc.tensor.transpose(psT[:, 0:n],
                                        snat[lo:hi, dt * 128:(dt + 1) * 128],
                                        id_s(lo, hi),
                                        tile_position=(lo, 0))
                    nc.vector.tensor_copy(s_bf[:, dt, lo:hi], psT[:, 0:n])
                for kt in range(KT):
                    a_mat(lo, hi, kt)
                alo = lo % 64
                nc.vector.tensor_copy(abf[h][alo:alo + n, :],
                                      psA[alo:alo + n, :])

            s_chunk(0)              # fills the PE while big8 streams
            for t in range(4):      # group 4 opens after close 0 frees a bank
                open_group(t)
            s_chunk(1)
            a_piece(0, 64)          # A samples 0-63 (only needs chunks 0-1)
            close_group(0)
            open_group(4)
            for t in range(1, 5):   # tiles 0-4 only touch samples 0-63
                close_group(t)
            s_chunk(2)
            for t in range(5, 9):
                open_group(t)
            # A samples 64-95; tiles 5-6 only touch samples 64-89
            a_piece(64, 96)
            close_group(5)
            open_group(9)
            close_group(6)
            s_chunk(3)
            a_piece(96, 128)        # A samples 96-127
            for t in range(7, 10):
                close_group(t)

    nc.compile()
    return nc


def _get_compiled():
    global _compiled
    if _compiled is None:
        _compiled = _build_bass()
    return _compiled


def _host_prep(inputs):
    """Shard + preprocess on host. Returns per-core input maps."""
    x = np.asarray(inputs["spatial_branch_feature_map"], dtype=np.float32)
    W_self = np.asarray(inputs["W_self"], dtype=np.float32)
    W_nbr = np.asarray(inputs["W_nbr"], dtype=np.float32)
    b = np.asarray(inputs["b"], dtype=np.float32)
    st = np.asarray(inputs["slicing_tensor"])
    op = np.asarray(inputs["object_pairs"])

    N = x.shape[0]
    n = NOBJ
    # exact replication of the reference's LUT-based row computation
    keys = st[:, 0].astype(np.int64) * (n * n) + st[:, 1].astype(np.int64) * n \
        + st[:, 2].astype(np.int64)
    lut = np.zeros(B * n * n, dtype=np.int64)
    lut[keys] = np.arange(N, dtype=np.int64)
    pmin = np.minimum(op[..., 0], op[..., 1]).astype(np.int64)
    pmax = np.maximum(op[..., 0], op[..., 1]).astype(np.int64)
    rel_keys = (np.arange(B, dtype=np.int64)[:, None] * (n * n)
                + pmin * n + pmax).reshape(-1)
    rows = lut[rel_keys]                      # [B*MAXR] global row index

    xg = x[rows]                              # [B*MAXR, D]
    # x: [NCORES, XCH, 128, RJ*D]; sbuf[p, j, :] = x_core[ch*896 + j*128 + p]
    x_bf = np.ascontiguousarray(
        x.astype(BF16).reshape(NCORES, XCH, RJ, 128, D)
        .transpose(0, 1, 3, 2, 4).reshape(NCORES, XCH, 128, RJ * D))
    # packed fp8 ws||xgT: big8[c][p, kt, 0:512] = W_self[kt*128+p, :],
    #   big8[c][p, kt, 512+t*128+m] = xg_c[t*128+m, kt*128+p]
    ws8 = np.ascontiguousarray(
        W_self.astype(FP8).reshape(KT, 128, D).transpose(1, 0, 2))  # [128,KT,D]
    xgT8 = (xg.astype(FP8).reshape(NCORES, MT, 128, KT, 128)
            .transpose(0, 4, 3, 1, 2))       # [NCORES, 128, KT, MT, 128]
    big8 = np.empty((NCORES, 128, KT, GW), dtype=FP8)
    big8[:, :, :, 0:D] = ws8[None]
    big8[:, :, :, D:] = xgT8.reshape(NCORES, 128, KT, MT * 128)
    big8 = big8.reshape(NCORES, 128, KT * GW)

    wn = np.ascontiguousarray(
        W_nbr.astype(BF16).reshape(KT, 128, D).transpose(1, 0, 2)
        .reshape(128, KT * D))
    ident = np.eye(128, dtype=BF16)

    # eTh[i<64, h*HM + m] = ((640h + m)//10 == 64h + i); row 64 = 1 (bias)
    eTh = np.zeros((65, 2 * HM), dtype=BF16)
    for h in range(2):
        m = np.arange(HM) + h * HM
        eTh[:64, h * HM:(h + 1) * HM] = (
            (m[None, :] // MAXR) == (np.arange(64)[:, None] + 64 * h)
        ).astype(BF16)
    eTh[64, :] = BF16(1.0)
    # shared one-hot block: g[p, j*SW + s] = ((j*128 + p)//NC2 == s)
    jj = np.arange(RJ * 128)
    g = (jj[:, None] // NC2 == np.arange(SW)[None, :]).astype(BF16)
    g = np.ascontiguousarray(
        g.reshape(RJ, 128, SW).transpose(1, 0, 2).reshape(128, RJ * SW))
    bias = b.astype(BF16).reshape(1, D)

    in_maps = []
    for c in range(NCORES):
        in_maps.append({
            "x": x_bf[c], "big8": big8[c], "g": g,
            "ident": ident, "wn": wn, "eTh": eTh, "bias": bias,
        })
    return in_maps


def run(inputs, trace=False):
    """Returns (full_output, BassKernelResults)."""
    from concourse.bass_utils import run_bass_kernel_spmd

    nc = _get_compiled()
    in_maps = _host_prep(inputs)
    res = run_bass_kernel_spmd(nc, in_maps, core_ids=list(range(NCORES)),
                               trace=trace)
    outs = []
    for r in res.results:
        o = np.asarray(r["out"]).reshape(128, MT, D)
        outs.append(o.transpose(1, 0, 2).reshape(ML, D))
    return np.concatenate(outs, axis=0).astype(np.float32), res


def kernel(**inputs) -> np.ndarray:
    out, _ = run(inputs, trace=False)
    return out
